# revision 15
# baseline (speedup 1.0000x reference)
"""Trainium2 Bass kernel for the MFA/MPPCA mixture log-likelihood problem.

Math: out[n,k] = PI[k] + logprob[n,k] with Sigma_k = A_k A_k^T + diag(D_k^2),
computed via Woodbury.  Everything involving only the small parameters
(MU, A, D, PI) is folded on the host into:

    out[n,k] = CONST[k] + x[n]·H[:,k] + (x[n]^2)·G[:,k] + sum_l (x[n]·Csc[:,k,l])^2

where (with iD = D^-2, B = iD*A, L = I + A^T B, iL = inv(L), R = chol(iL),
C0 = B R, e = R^T B^T MU):
    G   = -0.5 * iD^T                       (d, K)
    H   = (iD*MU)^T - C0 e                  (d, K)
    Csc = sqrt(0.5) * C0                    (d, K*l)
    CONST = PI - 0.5*(d log 2pi + logdet Sigma + MU^T iD MU) + 0.5 |e|^2

Device kernel (data-parallel over N on 8 cores), all big matmuls in fp8e4
with DoubleRow perf mode (2 fp8 weights per PE cell, 256-deep contraction):

  - x is sent as fp8 (x8); x^2 is sent as an exact hi+lo fp8 pair
    (x2h + x2l == x^2 to ~2^-9 relative), so the G path stays accurate.
  - wall8 (fp8, cols = [Csc k0:32 (320) | H (64) | Csc k32:64 (320)]) is the
    moving operand of 4 DoubleRow matmuls per 128-sample subtile with the
    x8 chunk-pairs stationary.
  - G path: G8 chunk-pairs stationary (cheap 128-col loads), hi/lo pairs
    moving -> psum_g[k, n]; ScalarE copies psum_g to SBUF adding CONST[k]
    as a per-partition bias; a PE transpose accumulates it back into the
    H region of the main psum, so no extra vector work is needed.
  - ScalarE squares the 640 factor projections (one 4D-AP activation),
    VectorE does the group-of-10 reduce and the single final add.
  - out is stored fp16 and upcast on the host.
"""
import math
import numpy as np
import ml_dtypes

N_TOTAL, K, D_FEAT, L_FAC = 131072, 64, 512, 10
N_CORES = 8
N_PER_CORE = N_TOTAL // N_CORES  # 16384

WALL_COLS = K + K * L_FAC        # 704
NG_HALF = K // 2                 # 32 groups per psum bank
CSC_HALF = NG_HALF * L_FAC       # 320
BANK_A_COLS = CSC_HALF + K       # 384 = [Csc_a | H]
GROUP = 1024                     # samples per DMA group (8 subtiles)

FP8 = ml_dtypes.float8_e4m3      # TRN fp8e4 (max +-240)


def host_prep(MU, A, D, PI):
    """Fold small-parameter math into matmul weights (float64 internally)."""
    MU64, A64, D64, PI64 = [np.asarray(v, np.float64) for v in (MU, A, D, PI)]
    Kc, d, l = A64.shape
    iD = D64 ** -2.0
    B = iD[..., None] * A64
    L = np.eye(l)[None] + np.einsum('kdl,kdm->klm', A64, B)
    sign, logdet_L = np.linalg.slogdet(L)
    log_det_Sigma = logdet_L - np.sum(np.log(iD), axis=1)
    iL = np.linalg.inv(L)
    R = np.linalg.cholesky(iL)                  # R @ R.T = iL
    C0 = np.einsum('kdl,klm->kdm', B, R)        # (K, d, l)
    bmu = np.einsum('kdl,kd->kl', B, MU64)
    e = np.einsum('klm,kl->km', R, bmu)         # (K, l)
    c1 = np.sum(iD * MU64 * MU64, axis=1)

    CONST = PI64 - 0.5 * (d * math.log(2.0 * math.pi) + log_det_Sigma + c1) \
        + 0.5 * np.sum(e * e, axis=1)
    G = (-0.5 * iD).T                                               # (d, K)
    H = (iD * MU64 - np.einsum('kdm,km->kd', C0, e)).T              # (d, K)
    Csc = (C0 * np.sqrt(0.5)).transpose(1, 0, 2).reshape(d, Kc * l)  # k-major

    wall = np.concatenate(
        [Csc[:, :CSC_HALF], H, Csc[:, CSC_HALF:]], axis=1).astype(FP8)
    # interleave chunk pairs: [p, pair, col, 2] so the DoubleRow moving
    # stream fetches both pair values from one 16B SBUF line (2 vals/cycle)
    wall_i = np.ascontiguousarray(
        wall.reshape(2, 2, 128, WALL_COLS).transpose(2, 0, 3, 1))
    g8 = G.astype(FP8)                                              # (d, K)
    cvec = CONST.astype(np.float32).reshape(K, 1)                   # (K, 1)
    ident = np.eye(K, dtype=np.float32)                             # (K, K)
    return wall_i, g8, cvec, ident


def build_nc(n_per_core=N_PER_CORE):
    """Build and compile the Bass module for one core (SPMD across 8)."""
    import concourse.bacc as bacc
    import concourse.tile as tile
    import concourse.mybir as mybir

    f32 = mybir.dt.float32
    f16 = mybir.dt.float16
    f8 = mybir.dt.float8e4
    DR = mybir.MatmulPerfMode.DoubleRow
    n_groups = n_per_core // GROUP
    n_sub_g = GROUP // 128       # 8 subtiles per DMA group
    assert n_per_core % GROUP == 0

    nc = bacc.Bacc("TRN2", target_bir_lowering=False, debug=False,
                   enable_asserts=False, num_devices=N_CORES)
    x8_dram = nc.dram_tensor("x8", (D_FEAT, n_per_core), f8, kind="ExternalInput")
    x2h_dram = nc.dram_tensor("x2h", (128, 2, n_per_core, 2), f8,
                              kind="ExternalInput")
    x2l_dram = nc.dram_tensor("x2l", (128, 2, n_per_core, 2), f8,
                              kind="ExternalInput")
    wall_dram = nc.dram_tensor("wall8i", (128, 2, WALL_COLS, 2), f8,
                               kind="ExternalInput")
    g_dram = nc.dram_tensor("g8", (D_FEAT, K), f8, kind="ExternalInput")
    c_dram = nc.dram_tensor("cvec", (K, 1), f32, kind="ExternalInput")
    i_dram = nc.dram_tensor("ident", (K, K), f32, kind="ExternalInput")
    out_dram = nc.dram_tensor("out", (n_per_core, K), f16, kind="ExternalOutput")

    x8_v = x8_dram.ap().rearrange("(c p) n -> p c n", p=128)       # [128,4,n]
    x2h_v = x2h_dram.ap()                                  # [128,2,n,2]
    x2l_v = x2l_dram.ap()
    wall_v = wall_dram.ap()                                # [128,2,704,2]
    g_v = g_dram.ap().rearrange("(c p) m -> p c m", p=128)         # [128,4,64]

    with tile.TileContext(nc) as tc:
        with (
            tc.tile_pool(name="wpool", bufs=1) as wpool,
            tc.tile_pool(name="xpool", bufs=2) as xpool,
            tc.tile_pool(name="spool", bufs=3) as spool,
            tc.tile_pool(name="opool", bufs=2) as opool,
            tc.tile_pool(name="ppool", bufs=3, space="PSUM") as ppool,
            tc.tile_pool(name="gpool", bufs=2, space="PSUM") as gpool,
        ):
            wall_sb = wpool.tile([128, 2, WALL_COLS, 2], f8)
            nc.sync.dma_start(out=wall_sb[:], in_=wall_v[:])
            g_sb = wpool.tile([128, 4, K], f8)
            nc.sync.dma_start(out=g_sb[:], in_=g_v[:])
            c_sb = wpool.tile([K, 1], f32)
            nc.sync.dma_start(out=c_sb[:], in_=c_dram.ap())
            ident_sb = wpool.tile([K, K], f32)
            nc.sync.dma_start(out=ident_sb[:], in_=i_dram.ap())

            # software pipeline: the tg->transpose->final-add tail of subtile
            # i runs while subtile i+1's matmuls stream, so the PE never
            # stalls on the scalar engine (keeps HAM at full clock).
            pending = None  # (ps, tg, red, out_slice) awaiting transpose+add

            def flush_pending():
                nonlocal pending
                if pending is None:
                    return
                ps_p, tg_p, red_p, out_slice = pending
                # transpose tg back to [n, k], accumulating into H region
                nc.tensor.matmul(
                    ps_p[:, CSC_HALF:BANK_A_COLS], tg_p[:], ident_sb[:],
                    start=False, stop=True, is_transpose=True,
                    skip_group_check=True)
                nc.vector.tensor_add(out_slice,
                                     ps_p[:, CSC_HALF:BANK_A_COLS], red_p[:])
                pending = None

            out_tiles = []
            for gi in range(n_groups):
                gsl = slice(gi * GROUP, (gi + 1) * GROUP)
                x8_sb = xpool.tile([128, 4, GROUP], f8, tag="x8")
                nc.sync.dma_start(out=x8_sb[:], in_=x8_v[:, :, gsl])
                x2h_sb = xpool.tile([128, 2, GROUP, 2], f8, tag="x2h")
                nc.sync.dma_start(out=x2h_sb[:], in_=x2h_v[:, :, gsl, :])
                x2l_sb = xpool.tile([128, 2, GROUP, 2], f8, tag="x2l")
                nc.sync.dma_start(out=x2l_sb[:], in_=x2l_v[:, :, gsl, :])

                out_t = opool.tile([128, n_sub_g, K], f16, tag="out")

                for j in range(n_sub_g):
                    nsl = slice(j * 128, (j + 1) * 128)
                    # main psum: bank0 = [Csc_a 320 | H 64], bank1 = [Csc_b 320]
                    ps = ppool.tile([128, 1024], f32, tag="ps")
                    ps_g = gpool.tile([K, 128], f32, tag="psg")

                    def dr(dst, cpair, cols, start, stop):
                        nc.tensor.matmul(
                            dst, x8_sb[:, cpair:cpair + 2, nsl],
                            wall_sb[:, cpair // 2, cols, :]
                            .rearrange("p n two -> p two n"),
                            start=start, stop=stop, perf_mode=DR,
                            skip_group_check=True)

                    # Csc + H: 4 DoubleRow matmuls (x8 chunk-pairs stationary)
                    dr(ps[:, 0:BANK_A_COLS], 0, slice(0, BANK_A_COLS), True, False)
                    dr(ps[:, 512:512 + CSC_HALF], 0,
                       slice(BANK_A_COLS, WALL_COLS), True, False)
                    dr(ps[:, 0:BANK_A_COLS], 2, slice(0, BANK_A_COLS), False, False)
                    dr(ps[:, 512:512 + CSC_HALF], 2,
                       slice(BANK_A_COLS, WALL_COLS), False, True)

                    # G path: G8 chunk-pairs stationary, hi/lo pairs moving
                    def gmm(xsb, cpair, start, stop):
                        nc.tensor.matmul(
                            ps_g[:], g_sb[:, cpair:cpair + 2, :],
                            xsb[:, cpair // 2, nsl, :]
                            .rearrange("p n two -> p two n"),
                            start=start, stop=stop, perf_mode=DR,
                            skip_group_check=True)

                    gmm(x2h_sb, 0, True, False)
                    gmm(x2l_sb, 0, False, False)
                    gmm(x2h_sb, 2, False, False)
                    gmm(x2l_sb, 2, False, True)

                    # retire the previous subtile's transpose + final add now,
                    # ahead of this subtile's scalar/vector ops on their queues
                    flush_pending()

                    # psum_g -> SBUF with CONST[k] folded in as bias (emitted
                    # before sq: it gates the next PE transpose)
                    tg = spool.tile([K, 128], f32, tag="tg")
                    nc.scalar.add(tg[:], ps_g[:], add=c_sb[:])

                    # squares of the 640 factor projections (both banks, 4D AP)
                    sq = spool.tile([128, 2, NG_HALF, L_FAC], f16, tag="sq")
                    nc.scalar.square(
                        sq[:],
                        ps.rearrange("p (b h) -> p b h", b=2)[:, :, 0:CSC_HALF]
                        .rearrange("p b (g t) -> p b g t", t=L_FAC))

                    red = spool.tile([128, K], f32, tag="red")
                    nc.vector.reduce_sum(red[:], sq[:], axis=mybir.AxisListType.X)

                    pending = (ps, tg, red, out_t[:, j, :])

                out_tiles.append((out_t, gsl))
                if len(out_tiles) > 1:
                    ot, osl = out_tiles.pop(0)
                    nc.sync.dma_start(
                        out=out_dram.ap()[osl, :].rearrange(
                            "(j p) k -> p j k", p=128),
                        in_=ot[:])

            flush_pending()
            for ot, osl in out_tiles:
                nc.sync.dma_start(
                    out=out_dram.ap()[osl, :].rearrange("(j p) k -> p j k", p=128),
                    in_=ot[:])

    nc.compile()
    return nc


_NC_CACHE = {}


def _get_nc(n_per_core=N_PER_CORE):
    if n_per_core not in _NC_CACHE:
        _NC_CACHE[n_per_core] = build_nc(n_per_core)
    return _NC_CACHE[n_per_core]


def _install_ntff_hook():
    """Provide the antenv.axon_hooks shim so trace=True can capture NTFFs."""
    import sys
    if "antenv.axon_hooks" in sys.modules:
        return
    import types
    import ctypes
    import contextlib

    so_path = "/opt/axon/libaxon_pjrt.so"
    lib = ctypes.CDLL(so_path)
    if not hasattr(lib, "axon_start_nrt_profile"):
        return
    lib.axon_start_nrt_profile.argtypes = [ctypes.POINTER(ctypes.c_int64), ctypes.c_size_t]
    lib.axon_start_nrt_profile.restype = ctypes.c_int64
    lib.axon_stop_nrt_profile.argtypes = [ctypes.c_char_p]
    lib.axon_stop_nrt_profile.restype = ctypes.c_int64

    @contextlib.contextmanager
    def _hook(output_dir, device_ids):
        import jax
        jax.devices()
        if device_ids:
            ids = (ctypes.c_int64 * len(device_ids))(*device_ids)
            rc = lib.axon_start_nrt_profile(ids, len(device_ids))
        else:
            rc = lib.axon_start_nrt_profile(None, 0)
        if rc != 0:
            raise RuntimeError(f"axon_start_nrt_profile rc={rc}")
        try:
            yield
        finally:
            n = lib.axon_stop_nrt_profile(str(output_dir).encode())
            print(f"ntff profile: {n} file(s) written to {output_dir}")

    mod = types.ModuleType("antenv.axon_hooks")
    mod.get_axon_ntff_profile_hook = lambda: _hook
    mod.set_axon_ntff_profile_hook = lambda h: None
    sys.modules["antenv.axon_hooks"] = mod


def kernel(x, MU, A, D, PI, trace=False):
    from concourse.bass_utils import run_bass_kernel_spmd
    if trace:
        try:
            _install_ntff_hook()
        except Exception as e:
            print(f"ntff hook install failed: {e}")
            trace = False

    x = np.asarray(x, np.float32)
    wall, g8, cvec, ident = host_prep(MU, A, D, PI)
    nc = _get_nc()

    def ileave(a):
        # (512, n) -> (128, pair, n, 2) chunk-pair interleave
        n = a.shape[1]
        return np.ascontiguousarray(
            a.reshape(2, 2, 128, n).transpose(2, 0, 3, 1))

    in_maps = []
    for c in range(N_CORES):
        xs = np.ascontiguousarray(x[c * N_PER_CORE:(c + 1) * N_PER_CORE, :].T)
        x8 = xs.astype(FP8)
        x2 = xs * xs
        x2h = x2.astype(FP8)
        x2l = (x2 - x2h.astype(np.float32)).astype(FP8)
        in_maps.append({"x8": x8, "x2h": ileave(x2h), "x2l": ileave(x2l),
                        "wall8i": wall, "g8": g8, "cvec": cvec, "ident": ident})

    res = run_bass_kernel_spmd(nc, in_maps, list(range(N_CORES)), trace=trace)
    out = np.concatenate(
        [res.results[c]["out"].astype(np.float32) for c in range(N_CORES)], axis=0)
    if trace:
        kernel.last_exec_time_ns = res.exec_time_ns
        kernel.last_results = res
    return out
